# revision 23
# baseline (speedup 1.0000x reference)
"""Trainium2 Bass kernel for AdaptedEnzymeModel (per-node MLP -> segment mean
pool -> graph MLP), SPMD over 8 NeuronCores.

Strategy
--------
* BatchNorm (eval) affines are folded into the adjacent Linear weights on the
  host; the device runs Linear+ReLU chains in bf16 (fp32 PSUM accumulate).
* The packed node dim is sharded at *graph boundaries*: core c gets graphs
  [512c, 512c+512), split into 4 groups of 128 graphs ("bins").  Every group is
  padded to a common node count G, so the 8 cores run one SPMD program with no
  collectives.
* Layers 1-5 are processed two 512-node chunks at a time using PE array tiling
  (64x64 mode): the two chunks run on disjoint array quadrants concurrently
  and land in one [128, 512] PSUM bank, so a single ReLU evacuates both.
  Contractions are zero-padded to K=64 (garbage rows are annihilated by
  zero weight rows).
* Layer 6 computes z6 = relu(z5^T @ W6 + b6) in [node, feat] layout (the
  transpose is free: z5 is the stationary operand); b6 enters via a PSUM
  prefill matmul (ones-row stationary, b6-row moving) so no extra vector op
  is needed.  All 4 subtiles accumulate into one [128, 512] PSUM bank and are
  evacuated by a single ReLU.
* Segment mean-pooling: the one-hot matrices OH[n, b] = (bin[n]==b) are built
  on the host and streamed in bf16 (DMA engines are otherwise idle); z6^T @ OH
  accumulates into a per-group PSUM bank, which is scaled by exact fp32
  1/count on the way out -> per-graph means in [feat, bin] layout.
* The tiny graph-level MLP (128->64->32->7) runs on-device in fp32 on the
  [128, 512] mean matrix; output is [7, 512] per core, reassembled on host.
* x is shipped as [64, 512] tiles (row r = nodes of 512-tile r); layer 1 is a
  selector matmul: lhsT column-block r holds W1 in row r and zeros elsewhere,
  so the matmul picks row r of the x tile.  All DMAs stay large and
  partition-spread.
"""

import numpy as np
import ml_dtypes
from contextlib import ExitStack

import concourse.bass as bass
import concourse.tile as tile
from concourse import bacc, mybir
from concourse.bass_utils import run_bass_kernel_spmd

NCORES = 8
GROUPS = 4          # bin-groups per core
BINS = 128          # graphs per group
NCLS = 7
EPS = 1e-5
F32 = mybir.dt.float32
BF16 = mybir.dt.bfloat16
NPBF = ml_dtypes.bfloat16
RELU = mybir.ActivationFunctionType.Relu
ALU = mybir.AluOpType

LAST_RESULT = None
_NC_CACHE = {}


def _ensure_ntff_hook():
    """bass_utils' trace path needs antenv.axon_hooks, which this image's
    antenv package lacks.  Register a shim backed by trn_agent_boot's ctypes
    NTFF driver so BASS_TRACE=1 yields exec_time_ns.  Degrades silently."""
    import sys
    import types
    try:
        import antenv
        if "antenv.axon_hooks" in sys.modules:
            return
        mod = types.ModuleType("antenv.axon_hooks")
        mod._hook = None
        mod.set_axon_ntff_profile_hook = lambda h: setattr(mod, "_hook", h)
        mod.get_axon_ntff_profile_hook = lambda: mod._hook
        sys.modules["antenv.axon_hooks"] = mod
        antenv.axon_hooks = mod
        from trn_agent_boot.trn_boot import _ntff_profile_via_ctypes
        mod._hook = _ntff_profile_via_ctypes("/opt/axon/libaxon_pjrt.so")
    except Exception:
        pass


_ensure_ntff_hook()


# ---------------------------------------------------------------- host math --
def _fold(p):
    """Fold eval-mode BN affines into adjacent linears. Returns dict of f32."""
    def aff(bn):
        g, b, m, v = bn[0], bn[1], bn[2], bn[3]
        s = g / np.sqrt(v + EPS)
        return s.astype(np.float32), (b - m * s).astype(np.float32)

    s1, t1 = aff(p["ne_bn1"]); s2, t2 = aff(p["ne_bn2"])
    sc1, tc1 = aff(p["cbn1"]); sc2, tc2 = aff(p["cbn2"])
    sf1, tf1 = aff(p["fbn1"]); sf2, tf2 = aff(p["fbn2"])
    f = {}
    f["W1"] = p["ne_w1"]; f["B1"] = p["ne_b1"]
    f["W2"] = s1[:, None] * p["ne_w2"]; f["B2"] = t1 @ p["ne_w2"] + p["ne_b2"]
    f["W3"] = s2[:, None] * p["c1a_w"]; f["B3"] = t2 @ p["c1a_w"] + p["c1a_b"]
    f["W4"] = p["c1b_w"];               f["B4"] = p["c1b_b"]
    f["W5"] = sc1[:, None] * p["c2a_w"]; f["B5"] = tc1 @ p["c2a_w"] + p["c2a_b"]
    f["W6"] = p["c2b_w"];               f["B6"] = p["c2b_b"]
    f["F1"] = sc2[:, None] * p["f1_w"]; f["F1B"] = tc2 @ p["f1_w"] + p["f1_b"]
    f["F2"] = sf1[:, None] * p["f2_w"]; f["F2B"] = tf1 @ p["f2_w"] + p["f2_b"]
    f["F3"] = sf2[:, None] * p["f3_w"]; f["F3B"] = tf2 @ p["f3_w"] + p["f3_b"]
    return {k: np.asarray(v, np.float32) for k, v in f.items()}


# bf16 const block: replicated/padded matmul weights + ones/bias rows
def _layout_bf():
    off, c = {}, 0
    for name, ncols in [("W2R", 64), ("W3R", 64), ("W4R", 64), ("W5R", 128),
                        ("W6", 128), ("ONES", 128), ("B6R4", 512)]:
        off[name] = c
        c += ncols
    return off, c


# f32 const block: biases (stacked to match packed PSUM layouts) + final mlp
def _layout_fp():
    off, c = {}, 0
    for name, ncols in [("F1", 64), ("F2", 32), ("F3", NCLS),
                        ("B1S", 1), ("B2S", 1), ("B3S", 1), ("B4S", 1),
                        ("B5", 1), ("F1B", 1), ("F2B", 1), ("F3B", 1)]:
        off[name] = c
        c += ncols
    return off, c


_OFFB, _CWB = _layout_bf()
_OFFF, _CWF = _layout_fp()


def _pack_consts(f):
    wb = np.zeros((128, _CWB), NPBF)

    def putb(name, arr):
        wb[:arr.shape[0], _OFFB[name]:_OFFB[name] + arr.shape[1]] = \
            arr.astype(NPBF)

    w2p = np.zeros((64, 64), np.float32)
    w2p[:32] = f["W2"]
    putb("W2R", np.tile(w2p, (2, 1)))
    putb("W3R", np.tile(f["W3"], (2, 1)))
    putb("W4R", np.tile(f["W4"], (2, 1)))
    putb("W5R", np.tile(f["W5"], (2, 1)))
    putb("W6", f["W6"])
    wb[0, _OFFB["ONES"]:_OFFB["ONES"] + 128] = NPBF(1.0)
    wb[0, _OFFB["B6R4"]:_OFFB["B6R4"] + 512] = np.tile(f["B6"].astype(NPBF), 4)

    wf = np.zeros((128, _CWF), np.float32)
    for k in ["F1", "F2", "F3"]:
        arr = f[k]
        wf[:arr.shape[0], _OFFF[k]:_OFFF[k] + arr.shape[1]] = arr
    wf[:, _OFFF["B1S"]] = np.tile(f["B1"], 4)
    wf[:, _OFFF["B2S"]] = np.tile(f["B2"], 2)
    wf[:, _OFFF["B3S"]] = np.tile(f["B3"], 2)
    wf[:, _OFFF["B4S"]] = np.tile(f["B4"], 2)
    wf[:128, _OFFF["B5"]] = f["B5"]
    for k, d in [("F1B", 64), ("F2B", 32), ("F3B", NCLS)]:
        wf[:d, _OFFF[k]] = f[k]
    return wb, wf


def _pack_sel(f):
    """Selector weights for layer 1: block r = [64, 64] with W1 in row r,
    cols 0:32 (cols 32:64 are zero so the full 64-partition PSUM quadrant is
    written, keeping garbage out of downstream K=64 contractions)."""
    sel = np.zeros((64, 64 * 64), NPBF)
    for r in range(64):
        sel[r, r * 64:r * 64 + 32] = f["W1"][0].astype(NPBF)
    return sel


# ------------------------------------------------------------- device build --
def _build(G):
    NT = G // 512            # 512-node tiles per group (even)
    NXG = -(-NT // 64)       # 64-row x tiles per group
    assert G % 1024 == 0

    nc = bacc.Bacc(None, target_bir_lowering=False)
    xs_d = nc.declare_dram_parameter("xs", [GROUPS, NXG, 64, 512], BF16, isOutput=False)
    ohs_d = nc.declare_dram_parameter("ohs", [GROUPS, NT, 128, 4 * BINS], BF16, isOutput=False)
    inv_d = nc.declare_dram_parameter("invbc", [128, GROUPS * BINS], F32, isOutput=False)
    wb_d = nc.declare_dram_parameter("wbf", [128, _CWB], BF16, isOutput=False)
    wf_d = nc.declare_dram_parameter("wfp", [128, _CWF], F32, isOutput=False)
    sel_d = nc.declare_dram_parameter("selc", [64, 64 * 64], BF16, isOutput=False)
    out_d = nc.declare_dram_parameter("out", [NCLS, GROUPS * BINS], F32, isOutput=True)

    with ExitStack() as ctx:
        tc = ctx.enter_context(tile.TileContext(nc))
        cpool = ctx.enter_context(tc.tile_pool(name="const", bufs=1))
        gpool = ctx.enter_context(tc.tile_pool(name="gacc", bufs=1))
        xpool = ctx.enter_context(tc.tile_pool(name="xg", bufs=3))
        zpool = ctx.enter_context(tc.tile_pool(name="z", bufs=4))
        spool = ctx.enter_context(tc.tile_pool(name="small", bufs=8))
        psP = ctx.enter_context(tc.tile_pool(name="psP", bufs=4, space="PSUM"))
        psB = ctx.enter_context(tc.tile_pool(name="psB", bufs=3, space="PSUM"))
        psG = ctx.enter_context(tc.tile_pool(name="psG", bufs=1, space="PSUM"))

        wbsb = cpool.tile([128, _CWB], BF16)
        nc.sync.dma_start(wbsb[:], wb_d[:])
        wfsb = cpool.tile([128, _CWF], F32)
        nc.sync.dma_start(wfsb[:], wf_d[:])
        invsb = cpool.tile([128, GROUPS * BINS], F32)
        nc.sync.dma_start(invsb[:], inv_d[:])
        selsb = cpool.tile([64, 64 * 64], BF16)
        nc.sync.dma_start(selsb[:], sel_d[:])

        def WB(name, k, m):
            o = _OFFB[name]
            return wbsb[0:k, o:o + m]

        def WF(name, k, m):
            o = _OFFF[name]
            return wfsb[0:k, o:o + m]

        w2r, w3r, w4r = WB("W2R", 128, 64), WB("W3R", 128, 64), WB("W4R", 128, 64)
        w5r, w6 = WB("W5R", 128, 128), WB("W6", 128, 128)
        ones = WB("ONES", 128, 128)
        b6r4 = WB("B6R4", 128, 512)
        f1, f2, f3 = WF("F1", 128, 64), WF("F2", 64, 32), WF("F3", 32, NCLS)
        b1s, b2s = WF("B1S", 128, 1), WF("B2S", 128, 1)
        b3s, b4s, b5 = WF("B3S", 128, 1), WF("B4S", 128, 1), WF("B5", 128, 1)
        f1b, f2b, f3b = WF("F1B", 64, 1), WF("F2B", 32, 1), WF("F3B", NCLS, 1)

        gsb = gpool.tile([128, GROUPS * BINS], F32)

        pgs = {}

        def z6phase(g, t, z5c, ohsb):
            """L6 (bias prefill + 4 accumulating matmuls) + one ReLU evac."""
            p6 = psB.tile([128, 512], F32, tag="bg")
            nc.tensor.matmul(p6[:], ones, b6r4, start=True, stop=False,
                             skip_group_check=True)
            for s in range(4):
                nc.tensor.matmul(p6[:, s * 128:(s + 1) * 128],
                                 z5c[:, s * 128:(s + 1) * 128], w6,
                                 start=False, stop=(s == 3),
                                 skip_group_check=True)
            z6q = spool.tile([128, 512], BF16, tag="z6q")
            nc.scalar.activation(z6q[:, 0:256], p6[:, 0:256], RELU)
            nc.vector.tensor_scalar(z6q[:, 256:512], p6[:, 256:512], 0.0,
                                    None, ALU.max)
            return (g, t, z6q, ohsb)

        def segphase(item):
            g, t, z6q, ohsb = item
            if g not in pgs:
                pgs[g] = psG.tile([128, BINS], F32, tag="pg", name=f"pg{g}")
            pg = pgs[g]
            for s in range(4):
                nc.tensor.matmul(pg[:], z6q[:, s * 128:(s + 1) * 128],
                                 ohsb[:, s * BINS:(s + 1) * BINS],
                                 start=(t == 0 and s == 0),
                                 stop=(t == NT - 1 and s == 3),
                                 skip_group_check=True)
            if t == NT - 1:
                # group done: scale sums by exact fp32 1/count -> means
                nc.vector.tensor_tensor(gsb[:, g * BINS:(g + 1) * BINS],
                                        pg[:],
                                        invsb[:, g * BINS:(g + 1) * BINS],
                                        ALU.mult)
                del pgs[g]

        xgs = {}
        prev = None
        pending = []
        for g in range(GROUPS):
            for i in range(NXG):
                xg = xpool.tile([64, 512], BF16, tag=f"xg{i}")
                nc.sync.dma_start(xg[:], xs_d[g, i])
                xgs[i] = xg

            for mi in range(NT // 2):
                tu, tv = 2 * mi, 2 * mi + 1
                # one-hot loads a full iteration ahead of their seg use
                ohu = spool.tile([128, 4 * BINS], BF16, tag="oh", name=f"ohu{g}_{mi}")
                nc.sync.dma_start(ohu[:], ohs_d[g, tu])
                ohv = spool.tile([128, 4 * BINS], BF16, tag="oh", name=f"ohv{g}_{mi}")
                nc.sync.dma_start(ohv[:], ohs_d[g, tv])

                # ---- L1: two col-tiled selector matmuls -> one bank ----
                p1 = psP.tile([128, 512], F32, tag="pk")
                ru, rv = tu % 64, tv % 64
                nc.tensor.matmul(p1[0:64, :], selsb[:, ru * 64:ru * 64 + 64],
                                 xgs[tu // 64][:], start=True, stop=True,
                                 tile_position=(0, 0))
                nc.tensor.matmul(p1[64:128, :], selsb[:, rv * 64:rv * 64 + 64],
                                 xgs[tv // 64][:], start=True, stop=True,
                                 tile_position=(0, 64))
                z1 = zpool.tile([128, 512], BF16, tag="z1")
                nc.scalar.activation(z1[:], p1[:], RELU, bias=b1s)

                # ---- L2 (K=32 padded to 64): diag tiles -> one bank ----
                p2 = psP.tile([128, 512], F32, tag="pk")
                nc.tensor.matmul(p2[0:64, :], w2r[0:64, :], z1[0:64, :],
                                 start=True, stop=True, tile_position=(0, 0))
                nc.tensor.matmul(p2[64:128, :], w2r[64:128, :], z1[64:128, :],
                                 start=True, stop=True, tile_position=(64, 64))
                z2 = zpool.tile([128, 512], BF16, tag="z2")
                nc.scalar.activation(z2[:], p2[:], RELU, bias=b2s)

                # ---- L3 ----
                p3 = psP.tile([128, 512], F32, tag="pk")
                nc.tensor.matmul(p3[0:64, :], w3r[0:64, :], z2[0:64, :],
                                 start=True, stop=True, tile_position=(0, 0))
                nc.tensor.matmul(p3[64:128, :], w3r[64:128, :], z2[64:128, :],
                                 start=True, stop=True, tile_position=(64, 64))
                z3 = zpool.tile([128, 512], BF16, tag="z3")
                nc.scalar.activation(z3[:], p3[:], RELU, bias=b3s)

                # ---- L4 ----
                p4 = psP.tile([128, 512], F32, tag="pk")
                nc.tensor.matmul(p4[0:64, :], w4r[0:64, :], z3[0:64, :],
                                 start=True, stop=True, tile_position=(0, 0))
                nc.tensor.matmul(p4[64:128, :], w4r[64:128, :], z3[64:128, :],
                                 start=True, stop=True, tile_position=(64, 64))
                z4 = zpool.tile([128, 512], BF16, tag="z4")
                nc.vector.tensor_scalar(z4[:, 0:256], p4[:, 0:256], b4s, 0.0,
                                        ALU.add, ALU.max)
                nc.scalar.activation(z4[:, 256:512], p4[:, 256:512], RELU,
                                     bias=b4s)

                # ---- deferred z6 stage of the previous macro-iteration ----
                # (its PE work hides the z4 evacuation latency)
                if prev is not None:
                    g_, tu_, tv_, z5u_, z5v_, ohu_, ohv_ = prev
                    pending.append(z6phase(g_, tu_, z5u_, ohu_))
                    pending.append(z6phase(g_, tv_, z5v_, ohv_))

                # ---- L5 (M=128): row tiles -> two banks ----
                p5u = psB.tile([128, 512], F32, tag="bg")
                nc.tensor.matmul(p5u[:], w5r[0:64, :], z4[0:64, :],
                                 start=True, stop=True, tile_position=(0, 0))
                p5v = psB.tile([128, 512], F32, tag="bg")
                nc.tensor.matmul(p5v[:], w5r[64:128, :], z4[64:128, :],
                                 start=True, stop=True, tile_position=(64, 0))
                z5u = zpool.tile([128, 512], BF16, tag="z5u")
                nc.vector.tensor_scalar(z5u[:], p5u[:], b5, 0.0, ALU.add, ALU.max)
                z5v = zpool.tile([128, 512], BF16, tag="z5v")
                nc.vector.tensor_scalar(z5v[:], p5v[:], b5, 0.0, ALU.add, ALU.max)

                # ---- seg deferred two macros back (hides z6q latency) ----
                while len(pending) > 2:
                    segphase(pending.pop(0))

                prev = (g, tu, tv, z5u, z5v, ohu, ohv)

        # flush the last macro-iteration
        g_, tu_, tv_, z5u_, z5v_, ohu_, ohv_ = prev
        pending.append(z6phase(g_, tu_, z5u_, ohu_))
        pending.append(z6phase(g_, tv_, z5v_, ohv_))
        for item in pending:
            segphase(item)

        pf1 = psP.tile([64, 512], F32, tag="pk")
        nc.tensor.matmul(pf1[:], f1, gsb[:], start=True, stop=True)
        a1 = zpool.tile([64, 512], F32, tag="a1")
        nc.scalar.activation(a1[:], pf1[:], RELU, bias=f1b)
        pf2 = psP.tile([32, 512], F32, tag="pk")
        nc.tensor.matmul(pf2[:], f2, a1[:], start=True, stop=True)
        a2 = zpool.tile([32, 512], F32, tag="a2")
        nc.scalar.activation(a2[:], pf2[:], RELU, bias=f2b)
        pf3 = psP.tile([NCLS, 512], F32, tag="pk")
        nc.tensor.matmul(pf3[:], f3, a2[:], start=True, stop=True)
        osb = zpool.tile([NCLS, 512], F32, tag="osb")
        nc.vector.tensor_scalar(osb[:], pf3[:], f3b, None, ALU.add)
        nc.sync.dma_start(out_d[:], osb[:])

    nc.compile()
    return nc


# -------------------------------------------------------------------- entry --
def kernel(**inputs):
    global LAST_RESULT
    x = np.asarray(inputs["x"], np.float32)
    batch = np.asarray(inputs["batch"], np.int32)
    B = int(np.asarray(inputs["num_graphs"]))
    assert B == NCORES * GROUPS * BINS, f"unexpected num_graphs {B}"

    params = {k: np.asarray(v, np.float32) for k, v in inputs.items()
              if k not in ("x", "batch", "num_graphs")}
    f = _fold(params)

    bounds = np.searchsorted(batch, np.arange(0, B + 1, BINS))
    seg = bounds[1:] - bounds[:-1]
    counts = np.bincount(batch, minlength=B)
    inv = (1.0 / np.maximum(counts, 1)).astype(np.float32)

    G = max(1024, int(-(-int(seg.max()) // 1024) * 1024))
    NT = G // 512
    NXG = -(-NT // 64)

    xs = np.zeros((NCORES, GROUPS, G), np.float32)
    bi = np.full((NCORES, GROUPS, G), -1.0, np.float32)
    for c in range(NCORES):
        for g in range(GROUPS):
            k = c * GROUPS + g
            s, e = int(bounds[k]), int(bounds[k + 1])
            n = e - s
            xs[c, g, :n] = x[s:e]
            bi[c, g, :n] = (batch[s:e] - k * BINS).astype(np.float32)
    xsp = np.zeros((NCORES, GROUPS, NXG * 64, 512), np.float32)
    xsp[:, :, :NT] = xs.reshape(NCORES, GROUPS, NT, 512)
    xsp = xsp.reshape(NCORES, GROUPS, NXG, 64, 512).astype(NPBF)

    # one-hot [core, group, tile, node, s*128+bin] built on host, bf16
    v = bi.reshape(NCORES, GROUPS, NT, 4, 128).transpose(0, 1, 2, 4, 3)
    valid = v >= 0
    cols = np.where(valid, v + np.arange(4, dtype=np.float32)[None, None, None,
                                                              None, :] * BINS,
                    0).astype(np.int64)
    ohs = np.zeros((NCORES, GROUPS, NT, 128, 4 * BINS), NPBF)
    np.put_along_axis(ohs, cols, valid.astype(NPBF), axis=-1)
    # per-core inv-count broadcast tile [128, GROUPS*BINS]
    invbc = np.ascontiguousarray(
        np.broadcast_to(inv.reshape(NCORES, GROUPS * BINS)[:, None, :],
                        (NCORES, 128, GROUPS * BINS)))

    wb, wf = _pack_consts(f)
    sel = _pack_sel(f)

    if G not in _NC_CACHE:
        _NC_CACHE[G] = _build(G)
    nc = _NC_CACHE[G]

    in_maps = [{"xs": xsp[c], "ohs": ohs[c], "invbc": invbc[c],
                "wbf": wb, "wfp": wf, "selc": sel} for c in range(NCORES)]
    res = run_bass_kernel_spmd(nc, in_maps, core_ids=list(range(NCORES)))
    LAST_RESULT = res
    outs = np.stack([res.results[i]["out"] for i in range(NCORES)])
    return np.ascontiguousarray(
        outs.transpose(0, 2, 1).reshape(B, NCLS)).astype(np.float32)


# revision 25
# speedup vs baseline: 1.1943x; 1.1943x over previous
"""Trainium2 Bass kernel for AdaptedEnzymeModel (per-node MLP -> segment mean
pool -> graph MLP), SPMD over 8 NeuronCores.

Strategy
--------
* BatchNorm (eval) affines are folded into the adjacent Linear weights on the
  host; the device runs Linear+ReLU chains in bf16 (fp32 PSUM accumulate).
* The packed node dim is sharded at *graph boundaries*: core c gets graphs
  [512c, 512c+512), split into 4 groups of 128 graphs ("bins").  Every group is
  padded to a common node count G, so the 8 cores run one SPMD program with no
  collectives.
* Layers 1-5 are processed two 512-node chunks at a time using PE array tiling
  (64x64 mode): the two chunks run on disjoint array quadrants concurrently
  and land in one [128, 512] PSUM bank, so a single ReLU evacuates both.
  Contractions are zero-padded to K=64 (garbage rows are annihilated by
  zero weight rows).
* Layer 6 computes z6 = relu(z5^T @ W6 + b6) in [node, feat] layout (the
  transpose is free: z5 is the stationary operand); b6 enters via a PSUM
  prefill matmul (ones-row stationary, b6-row moving) so no extra vector op
  is needed.  All 4 subtiles accumulate into one [128, 512] PSUM bank and are
  evacuated by a single ReLU.
* Segment mean-pooling: the one-hot matrices OH[n, b] = (bin[n]==b) are built
  on the host and streamed in bf16 (DMA engines are otherwise idle); z6^T @ OH
  accumulates into a per-group PSUM bank, which is scaled by exact fp32
  1/count on the way out -> per-graph means in [feat, bin] layout.
* The tiny graph-level MLP (128->64->32->7) runs on-device in fp32 on the
  [128, 512] mean matrix; output is [7, 512] per core, reassembled on host.
* x is shipped as [64, 512] tiles (row r = nodes of 512-tile r); layer 1 is a
  selector matmul: lhsT column-block r holds W1 in row r and zeros elsewhere,
  so the matmul picks row r of the x tile.  All DMAs stay large and
  partition-spread.
"""

import numpy as np
import ml_dtypes
from contextlib import ExitStack

import concourse.bass as bass
import concourse.tile as tile
from concourse import bacc, mybir
from concourse.bass_utils import run_bass_kernel_spmd

NCORES = 8
GROUPS = 4          # bin-groups per core
BINS = 128          # graphs per group
NCLS = 7
EPS = 1e-5
F32 = mybir.dt.float32
BF16 = mybir.dt.bfloat16
NPBF = ml_dtypes.bfloat16
RELU = mybir.ActivationFunctionType.Relu
ALU = mybir.AluOpType

LAST_RESULT = None
_NC_CACHE = {}


def _ensure_ntff_hook():
    """bass_utils' trace path needs antenv.axon_hooks, which this image's
    antenv package lacks.  Register a shim backed by trn_agent_boot's ctypes
    NTFF driver so BASS_TRACE=1 yields exec_time_ns.  Degrades silently."""
    import sys
    import types
    try:
        import antenv
        if "antenv.axon_hooks" in sys.modules:
            return
        mod = types.ModuleType("antenv.axon_hooks")
        mod._hook = None
        mod.set_axon_ntff_profile_hook = lambda h: setattr(mod, "_hook", h)
        mod.get_axon_ntff_profile_hook = lambda: mod._hook
        sys.modules["antenv.axon_hooks"] = mod
        antenv.axon_hooks = mod
        from trn_agent_boot.trn_boot import _ntff_profile_via_ctypes
        mod._hook = _ntff_profile_via_ctypes("/opt/axon/libaxon_pjrt.so")
    except Exception:
        pass


_ensure_ntff_hook()


# ---------------------------------------------------------------- host math --
def _fold(p):
    """Fold eval-mode BN affines into adjacent linears. Returns dict of f32."""
    def aff(bn):
        g, b, m, v = bn[0], bn[1], bn[2], bn[3]
        s = g / np.sqrt(v + EPS)
        return s.astype(np.float32), (b - m * s).astype(np.float32)

    s1, t1 = aff(p["ne_bn1"]); s2, t2 = aff(p["ne_bn2"])
    sc1, tc1 = aff(p["cbn1"]); sc2, tc2 = aff(p["cbn2"])
    sf1, tf1 = aff(p["fbn1"]); sf2, tf2 = aff(p["fbn2"])
    f = {}
    f["W1"] = p["ne_w1"]; f["B1"] = p["ne_b1"]
    f["W2"] = s1[:, None] * p["ne_w2"]; f["B2"] = t1 @ p["ne_w2"] + p["ne_b2"]
    f["W3"] = s2[:, None] * p["c1a_w"]; f["B3"] = t2 @ p["c1a_w"] + p["c1a_b"]
    f["W4"] = p["c1b_w"];               f["B4"] = p["c1b_b"]
    f["W5"] = sc1[:, None] * p["c2a_w"]; f["B5"] = tc1 @ p["c2a_w"] + p["c2a_b"]
    f["W6"] = p["c2b_w"];               f["B6"] = p["c2b_b"]
    f["F1"] = sc2[:, None] * p["f1_w"]; f["F1B"] = tc2 @ p["f1_w"] + p["f1_b"]
    f["F2"] = sf1[:, None] * p["f2_w"]; f["F2B"] = tf1 @ p["f2_w"] + p["f2_b"]
    f["F3"] = sf2[:, None] * p["f3_w"]; f["F3B"] = tf2 @ p["f3_w"] + p["f3_b"]
    return {k: np.asarray(v, np.float32) for k, v in f.items()}


# bf16 const block: replicated/padded matmul weights + ones/bias rows
def _layout_bf():
    off, c = {}, 0
    for name, ncols in [("W2R", 64), ("W3R", 64), ("W4R", 64), ("W5R", 128),
                        ("W6", 128), ("ONES", 128), ("B6R4", 512)]:
        off[name] = c
        c += ncols
    return off, c


# f32 const block: biases (stacked to match packed PSUM layouts) + final mlp
def _layout_fp():
    off, c = {}, 0
    for name, ncols in [("F1", 64), ("F2", 32), ("F3", NCLS),
                        ("B1S", 1), ("B2S", 1), ("B3S", 1), ("B4S", 1),
                        ("B5", 1), ("F1B", 1), ("F2B", 1), ("F3B", 1)]:
        off[name] = c
        c += ncols
    return off, c


_OFFB, _CWB = _layout_bf()
_OFFF, _CWF = _layout_fp()


def _pack_consts(f):
    wb = np.zeros((128, _CWB), NPBF)

    def putb(name, arr):
        wb[:arr.shape[0], _OFFB[name]:_OFFB[name] + arr.shape[1]] = \
            arr.astype(NPBF)

    w2p = np.zeros((64, 64), np.float32)
    w2p[:32] = f["W2"]
    putb("W2R", np.tile(w2p, (2, 1)))
    putb("W3R", np.tile(f["W3"], (2, 1)))
    putb("W4R", np.tile(f["W4"], (2, 1)))
    putb("W5R", np.tile(f["W5"], (2, 1)))
    putb("W6", f["W6"])
    wb[0, _OFFB["ONES"]:_OFFB["ONES"] + 128] = NPBF(1.0)
    wb[0, _OFFB["B6R4"]:_OFFB["B6R4"] + 512] = np.tile(f["B6"].astype(NPBF), 4)

    wf = np.zeros((128, _CWF), np.float32)
    for k in ["F1", "F2", "F3"]:
        arr = f[k]
        wf[:arr.shape[0], _OFFF[k]:_OFFF[k] + arr.shape[1]] = arr
    wf[:, _OFFF["B1S"]] = np.tile(f["B1"], 4)
    wf[:, _OFFF["B2S"]] = np.tile(f["B2"], 2)
    wf[:, _OFFF["B3S"]] = np.tile(f["B3"], 2)
    wf[:, _OFFF["B4S"]] = np.tile(f["B4"], 2)
    wf[:128, _OFFF["B5"]] = f["B5"]
    for k, d in [("F1B", 64), ("F2B", 32), ("F3B", NCLS)]:
        wf[:d, _OFFF[k]] = f[k]
    return wb, wf


def _pack_sel(f):
    """Selector weights for layer 1: block r = [64, 64] with W1 in row r,
    cols 0:32 (cols 32:64 are zero so the full 64-partition PSUM quadrant is
    written, keeping garbage out of downstream K=64 contractions)."""
    sel = np.zeros((64, 64 * 64), NPBF)
    for r in range(64):
        sel[r, r * 64:r * 64 + 32] = f["W1"][0].astype(NPBF)
    return sel


# ------------------------------------------------------------- device build --
def _build(G):
    NT = G // 512            # 512-node tiles per group (even)
    NXG = -(-NT // 64)       # 64-row x tiles per group
    assert G % 1024 == 0

    nc = bacc.Bacc(None, target_bir_lowering=False)
    xs_d = nc.declare_dram_parameter("xs", [GROUPS, NXG, 64, 512], BF16, isOutput=False)
    ohs_d = nc.declare_dram_parameter("ohs", [GROUPS, NT, 128, 4 * BINS], BF16, isOutput=False)
    inv_d = nc.declare_dram_parameter("invbc", [128, GROUPS * BINS], F32, isOutput=False)
    wb_d = nc.declare_dram_parameter("wbf", [128, _CWB], BF16, isOutput=False)
    wf_d = nc.declare_dram_parameter("wfp", [128, _CWF], F32, isOutput=False)
    sel_d = nc.declare_dram_parameter("selc", [64, 64 * 64], BF16, isOutput=False)
    out_d = nc.declare_dram_parameter("out", [NCLS, GROUPS * BINS], F32, isOutput=True)

    with ExitStack() as ctx:
        tc = ctx.enter_context(tile.TileContext(nc))
        cpool = ctx.enter_context(tc.tile_pool(name="const", bufs=1))
        gpool = ctx.enter_context(tc.tile_pool(name="gacc", bufs=1))
        xpool = ctx.enter_context(tc.tile_pool(name="xg", bufs=3))
        zpool = ctx.enter_context(tc.tile_pool(name="z", bufs=4))
        spool = ctx.enter_context(tc.tile_pool(name="small", bufs=12))
        psP = ctx.enter_context(tc.tile_pool(name="psP", bufs=4, space="PSUM"))
        psB = ctx.enter_context(tc.tile_pool(name="psB", bufs=3, space="PSUM"))
        psG = ctx.enter_context(tc.tile_pool(name="psG", bufs=1, space="PSUM"))

        wbsb = cpool.tile([128, _CWB], BF16)
        nc.sync.dma_start(wbsb[:], wb_d[:])
        wfsb = cpool.tile([128, _CWF], F32)
        nc.sync.dma_start(wfsb[:], wf_d[:])
        invsb = cpool.tile([128, GROUPS * BINS], F32)
        nc.sync.dma_start(invsb[:], inv_d[:])
        selsb = cpool.tile([64, 64 * 64], BF16)
        nc.sync.dma_start(selsb[:], sel_d[:])

        def WB(name, k, m):
            o = _OFFB[name]
            return wbsb[0:k, o:o + m]

        def WF(name, k, m):
            o = _OFFF[name]
            return wfsb[0:k, o:o + m]

        w2r, w3r, w4r = WB("W2R", 128, 64), WB("W3R", 128, 64), WB("W4R", 128, 64)
        w5r, w6 = WB("W5R", 128, 128), WB("W6", 128, 128)
        ones = WB("ONES", 128, 128)
        b6r4 = WB("B6R4", 128, 512)
        f1, f2, f3 = WF("F1", 128, 64), WF("F2", 64, 32), WF("F3", 32, NCLS)
        b1s, b2s = WF("B1S", 128, 1), WF("B2S", 128, 1)
        b3s, b4s, b5 = WF("B3S", 128, 1), WF("B4S", 128, 1), WF("B5", 128, 1)
        f1b, f2b, f3b = WF("F1B", 64, 1), WF("F2B", 32, 1), WF("F3B", NCLS, 1)

        gsb = gpool.tile([128, GROUPS * BINS], F32)

        pgs = {}

        def z6phase(g, t, z5c, ohsb):
            """L6 (bias prefill + 4 accumulating matmuls) + one ReLU evac."""
            p6 = psB.tile([128, 512], F32, tag="bg")
            nc.tensor.matmul(p6[:], ones, b6r4, start=True, stop=False,
                             skip_group_check=True)
            for s in range(4):
                nc.tensor.matmul(p6[:, s * 128:(s + 1) * 128],
                                 z5c[:, s * 128:(s + 1) * 128], w6,
                                 start=False, stop=(s == 3),
                                 skip_group_check=True)
            z6q = spool.tile([128, 512], BF16, tag="z6q")
            if t % 2 == 0:
                nc.scalar.activation(z6q[:], p6[:], RELU)
            else:
                nc.vector.tensor_scalar(z6q[:], p6[:], 0.0, None, ALU.max)
            return (g, t, z6q, ohsb)

        def segphase(item):
            g, t, z6q, ohsb = item
            if g not in pgs:
                pgs[g] = psG.tile([128, BINS], F32, tag="pg", name=f"pg{g}")
            pg = pgs[g]
            for s in range(4):
                nc.tensor.matmul(pg[:], z6q[:, s * 128:(s + 1) * 128],
                                 ohsb[:, s * BINS:(s + 1) * BINS],
                                 start=(t == 0 and s == 0),
                                 stop=(t == NT - 1 and s == 3),
                                 skip_group_check=True)
            if t == NT - 1:
                # group done: scale sums by exact fp32 1/count -> means
                nc.vector.tensor_tensor(gsb[:, g * BINS:(g + 1) * BINS],
                                        pg[:],
                                        invsb[:, g * BINS:(g + 1) * BINS],
                                        ALU.mult)
                del pgs[g]

        xgs = {}
        prev = None
        pending = []
        for g in range(GROUPS):
            for i in range(NXG):
                xg = xpool.tile([64, 512], BF16, tag=f"xg{i}")
                nc.sync.dma_start(xg[:], xs_d[g, i])
                xgs[i] = xg

            for mi in range(NT // 2):
                tu, tv = 2 * mi, 2 * mi + 1
                # one-hot loads a full iteration ahead of their seg use
                ohu = spool.tile([128, 4 * BINS], BF16, tag="oh", name=f"ohu{g}_{mi}")
                nc.sync.dma_start(ohu[:], ohs_d[g, tu])
                ohv = spool.tile([128, 4 * BINS], BF16, tag="oh", name=f"ohv{g}_{mi}")
                nc.sync.dma_start(ohv[:], ohs_d[g, tv])

                # ---- L1: two col-tiled selector matmuls -> one bank ----
                p1 = psP.tile([128, 512], F32, tag="pk")
                ru, rv = tu % 64, tv % 64
                nc.tensor.matmul(p1[0:64, :], selsb[:, ru * 64:ru * 64 + 64],
                                 xgs[tu // 64][:], start=True, stop=True,
                                 tile_position=(0, 0))
                nc.tensor.matmul(p1[64:128, :], selsb[:, rv * 64:rv * 64 + 64],
                                 xgs[tv // 64][:], start=True, stop=True,
                                 tile_position=(0, 64))
                z1 = zpool.tile([128, 512], BF16, tag="z1")
                nc.scalar.activation(z1[:], p1[:], RELU, bias=b1s)

                # ---- L2 (K=32 padded to 64): diag tiles -> one bank ----
                p2 = psP.tile([128, 512], F32, tag="pk")
                nc.tensor.matmul(p2[0:64, :], w2r[0:64, :], z1[0:64, :],
                                 start=True, stop=True, tile_position=(0, 0))
                nc.tensor.matmul(p2[64:128, :], w2r[64:128, :], z1[64:128, :],
                                 start=True, stop=True, tile_position=(64, 64))
                z2 = zpool.tile([128, 512], BF16, tag="z2")
                nc.scalar.activation(z2[:], p2[:], RELU, bias=b2s)

                # ---- L3 ----
                p3 = psP.tile([128, 512], F32, tag="pk")
                nc.tensor.matmul(p3[0:64, :], w3r[0:64, :], z2[0:64, :],
                                 start=True, stop=True, tile_position=(0, 0))
                nc.tensor.matmul(p3[64:128, :], w3r[64:128, :], z2[64:128, :],
                                 start=True, stop=True, tile_position=(64, 64))
                z3 = zpool.tile([128, 512], BF16, tag="z3")
                nc.scalar.activation(z3[:], p3[:], RELU, bias=b3s)

                # ---- L4 ----
                p4 = psP.tile([128, 512], F32, tag="pk")
                nc.tensor.matmul(p4[0:64, :], w4r[0:64, :], z3[0:64, :],
                                 start=True, stop=True, tile_position=(0, 0))
                nc.tensor.matmul(p4[64:128, :], w4r[64:128, :], z3[64:128, :],
                                 start=True, stop=True, tile_position=(64, 64))
                z4 = zpool.tile([128, 512], BF16, tag="z4")
                nc.vector.tensor_scalar(z4[:, 0:256], p4[:, 0:256], b4s, 0.0,
                                        ALU.add, ALU.max)
                nc.scalar.activation(z4[:, 256:512], p4[:, 256:512], RELU,
                                     bias=b4s)

                # ---- deferred z6 stage of the previous macro-iteration ----
                # (its PE work hides the z4 evacuation latency)
                if prev is not None:
                    g_, tu_, tv_, z5u_, z5v_, ohu_, ohv_ = prev
                    pending.append(z6phase(g_, tu_, z5u_, ohu_))
                    pending.append(z6phase(g_, tv_, z5v_, ohv_))

                # ---- L5 (M=128): row tiles -> two banks ----
                p5u = psB.tile([128, 512], F32, tag="bg")
                nc.tensor.matmul(p5u[:], w5r[0:64, :], z4[0:64, :],
                                 start=True, stop=True, tile_position=(0, 0))
                p5v = psB.tile([128, 512], F32, tag="bg")
                nc.tensor.matmul(p5v[:], w5r[64:128, :], z4[64:128, :],
                                 start=True, stop=True, tile_position=(64, 0))
                z5u = zpool.tile([128, 512], BF16, tag="z5u")
                nc.vector.tensor_scalar(z5u[:], p5u[:], b5, 0.0, ALU.add, ALU.max)
                z5v = zpool.tile([128, 512], BF16, tag="z5v")
                nc.vector.tensor_scalar(z5v[:], p5v[:], b5, 0.0, ALU.add, ALU.max)

                # ---- seg deferred two macros back (hides z6q latency) ----
                while len(pending) > 4:
                    segphase(pending.pop(0))

                prev = (g, tu, tv, z5u, z5v, ohu, ohv)

        # flush the last macro-iteration
        g_, tu_, tv_, z5u_, z5v_, ohu_, ohv_ = prev
        pending.append(z6phase(g_, tu_, z5u_, ohu_))
        pending.append(z6phase(g_, tv_, z5v_, ohv_))
        for item in pending:
            segphase(item)

        pf1 = psP.tile([64, 512], F32, tag="pk")
        nc.tensor.matmul(pf1[:], f1, gsb[:], start=True, stop=True)
        a1 = zpool.tile([64, 512], F32, tag="a1")
        nc.scalar.activation(a1[:], pf1[:], RELU, bias=f1b)
        pf2 = psP.tile([32, 512], F32, tag="pk")
        nc.tensor.matmul(pf2[:], f2, a1[:], start=True, stop=True)
        a2 = zpool.tile([32, 512], F32, tag="a2")
        nc.scalar.activation(a2[:], pf2[:], RELU, bias=f2b)
        pf3 = psP.tile([NCLS, 512], F32, tag="pk")
        nc.tensor.matmul(pf3[:], f3, a2[:], start=True, stop=True)
        osb = zpool.tile([NCLS, 512], F32, tag="osb")
        nc.vector.tensor_scalar(osb[:], pf3[:], f3b, None, ALU.add)
        nc.sync.dma_start(out_d[:], osb[:])

    nc.compile()
    return nc


# -------------------------------------------------------------------- entry --
def kernel(**inputs):
    global LAST_RESULT
    x = np.asarray(inputs["x"], np.float32)
    batch = np.asarray(inputs["batch"], np.int32)
    B = int(np.asarray(inputs["num_graphs"]))
    assert B == NCORES * GROUPS * BINS, f"unexpected num_graphs {B}"

    params = {k: np.asarray(v, np.float32) for k, v in inputs.items()
              if k not in ("x", "batch", "num_graphs")}
    f = _fold(params)

    bounds = np.searchsorted(batch, np.arange(0, B + 1, BINS))
    seg = bounds[1:] - bounds[:-1]
    counts = np.bincount(batch, minlength=B)
    inv = (1.0 / np.maximum(counts, 1)).astype(np.float32)

    G = max(1024, int(-(-int(seg.max()) // 1024) * 1024))
    NT = G // 512
    NXG = -(-NT // 64)

    xs = np.zeros((NCORES, GROUPS, G), np.float32)
    bi = np.full((NCORES, GROUPS, G), -1.0, np.float32)
    for c in range(NCORES):
        for g in range(GROUPS):
            k = c * GROUPS + g
            s, e = int(bounds[k]), int(bounds[k + 1])
            n = e - s
            xs[c, g, :n] = x[s:e]
            bi[c, g, :n] = (batch[s:e] - k * BINS).astype(np.float32)
    xsp = np.zeros((NCORES, GROUPS, NXG * 64, 512), np.float32)
    xsp[:, :, :NT] = xs.reshape(NCORES, GROUPS, NT, 512)
    xsp = xsp.reshape(NCORES, GROUPS, NXG, 64, 512).astype(NPBF)

    # one-hot [core, group, tile, node, s*128+bin] built on host, bf16
    v = bi.reshape(NCORES, GROUPS, NT, 4, 128).transpose(0, 1, 2, 4, 3)
    valid = v >= 0
    cols = np.where(valid, v + np.arange(4, dtype=np.float32)[None, None, None,
                                                              None, :] * BINS,
                    0).astype(np.int64)
    ohs = np.zeros((NCORES, GROUPS, NT, 128, 4 * BINS), NPBF)
    np.put_along_axis(ohs, cols, valid.astype(NPBF), axis=-1)
    # per-core inv-count broadcast tile [128, GROUPS*BINS]
    invbc = np.ascontiguousarray(
        np.broadcast_to(inv.reshape(NCORES, GROUPS * BINS)[:, None, :],
                        (NCORES, 128, GROUPS * BINS)))

    wb, wf = _pack_consts(f)
    sel = _pack_sel(f)

    if G not in _NC_CACHE:
        _NC_CACHE[G] = _build(G)
    nc = _NC_CACHE[G]

    in_maps = [{"xs": xsp[c], "ohs": ohs[c], "invbc": invbc[c],
                "wbf": wb, "wfp": wf, "selc": sel} for c in range(NCORES)]
    res = run_bass_kernel_spmd(nc, in_maps, core_ids=list(range(NCORES)))
    LAST_RESULT = res
    outs = np.stack([res.results[i]["out"] for i in range(NCORES)])
    return np.ascontiguousarray(
        outs.transpose(0, 2, 1).reshape(B, NCLS)).astype(np.float32)
